# revision 34
# baseline (speedup 1.0000x reference)
"""AgglutinativeAttention Trainium2 kernel.

Full inputs in, full output out. Sharding: 8 cores = (batch b in 0..3) x
(head-group g in 0..1). Each core computes, for its batch b and its 8 heads:
  qT/kT = (x @ W{q,k}[:, gF:(g+1)F])^T   [512 feat, 1024 tok]
  v     =  x @ Wv[:, gF:(g+1)F]          [1024 tok, 512 feat] (+ones col/head)
  per head: sT = scores transposed [j, i] via fp8 DoubleRow matmuls (dithered
  dual quantization, see the qT/kT comment), morpho verb bias via an ebT
  elementwise factor + per-partition activation bias (col bias),
  pT = exp(scale*sT + cb) * ebT, oT = v_aug^T @ pT with a ones row giving the
  softmax denominator, divide, then partial z = o @ Wo[gF:(g+1)F, :].
  Host sums the two per-batch partials + bo.

x and the q/k/v weights travel as fp8 hi+residual pairs (split-fp8
DoubleRow projections at 0.75x the bf16 PE cost), wo and z as bf16. DMA
order is arranged so the first v-proj matmul is gated on a few hundred KB,
not 7MB; warm-up matmuls burn the PE p-state ramp during the initial DMA
wait.
"""

import numpy as np
import ml_dtypes
from contextlib import ExitStack

import concourse.bass as bass
import concourse.mybir as mybir
import concourse.tile as tile
from concourse import bacc
from concourse.bass_utils import run_bass_kernel_spmd

B, S, H = 4, 1024, 1024
NH, HD = 16, 64
G = 2                 # head groups (tensor-parallel factor per batch)
F = H // G            # 512 features per core
HPC = NH // G         # 8 heads per core
SCALE = 1.0 / np.sqrt(HD)
VERB_BIAS, ROOT_BIAS, SUFFIX_BIAS = 2.0, 1.5, 1.2
BIG = np.float32(1e9)

f32 = mybir.dt.float32
f32r = mybir.dt.float32r
bf16 = mybir.dt.bfloat16
i32 = mybir.dt.int32

P = 128
KC = H // P           # 8 contraction chunks for projections
TC = S // P           # 8 token chunks of 128
IC = S // 512         # 2 chunks of 512 (matmul free dim)
FC = F // P           # 4 feature chunks per core

_COMPILED = None


def _build():
    nc = bacc.Bacc("TRN2", target_bir_lowering=False, debug=False, num_devices=8)

    fp8d = mybir.dt.float8e4
    x_d = nc.dram_tensor("x", [H, S], fp8d, kind="ExternalInput").ap()
    xr_d = nc.dram_tensor("xr", [H, S], fp8d, kind="ExternalInput").ap()
    wq_d = nc.dram_tensor("wq", [H, F], fp8d, kind="ExternalInput").ap()
    wqr_d = nc.dram_tensor("wqr", [H, F], fp8d, kind="ExternalInput").ap()
    wk_d = nc.dram_tensor("wk", [H, F], fp8d, kind="ExternalInput").ap()
    wkr_d = nc.dram_tensor("wkr", [H, F], fp8d, kind="ExternalInput").ap()
    wv_d = nc.dram_tensor("wv", [H, F], fp8d, kind="ExternalInput").ap()
    wvr_d = nc.dram_tensor("wvr", [H, F], fp8d, kind="ExternalInput").ap()
    wo_d = nc.dram_tensor("wo", [F, H], bf16, kind="ExternalInput").ap()
    bqs_d = nc.dram_tensor("bqs", [F], f32, kind="ExternalInput").ap()
    bk_d = nc.dram_tensor("bk", [F], f32, kind="ExternalInput").ap()
    bv_d = nc.dram_tensor("bv", [F], f32, kind="ExternalInput").ap()
    nearf_d = nc.dram_tensor("nearf", [S], f32, kind="ExternalInput").ap()
    cb_d = nc.dram_tensor("cb", [S], f32, kind="ExternalInput").ap()
    z_d = nc.dram_tensor("z", [S, H], bf16, kind="ExternalOutput").ap()

    with tile.TileContext(nc) as tc, ExitStack() as ctx:
        const = ctx.enter_context(tc.tile_pool(name="const", bufs=1))
        big = ctx.enter_context(tc.tile_pool(name="big", bufs=1))
        ppool = ctx.enter_context(tc.tile_pool(name="ppool", bufs=6))
        rlpool = ctx.enter_context(tc.tile_pool(name="rlpool", bufs=2))
        odpool = ctx.enter_context(tc.tile_pool(name="odpool", bufs=2))
        zpool = ctx.enter_context(tc.tile_pool(name="zpool", bufs=6))
        ps_q = ctx.enter_context(tc.tile_pool(name="ps_q", bufs=2, space="PSUM"))
        ps_s = ctx.enter_context(tc.tile_pool(name="ps_s", bufs=2, space="PSUM"))
        ps_o = ctx.enter_context(tc.tile_pool(name="ps_o", bufs=1, space="PSUM"))

        # ---- small constants via SWDGE (gpsimd) — keeps HWDGE free for the
        # wv/x stream that gates the first matmul. Only the two needed early
        # (bv for the first v eviction, nearf for the ohst chain) go first;
        # the rest are emitted after the ohst loop so their transfers don't
        # sit ahead of the wv/x stream on the DMA engines ----
        near_row = const.tile([1, S], f32, tag="near_row")
        bv_row = const.tile([1, F], f32, tag="bv_row")
        nc.gpsimd.dma_start(bv_row[:], bv_d[None, :])
        nc.gpsimd.dma_start(near_row[:], nearf_d[None, :])
        cb_sb = const.tile([P, TC], f32, tag="cb_sb")
        bq_sb = const.tile([P, FC], f32, tag="bq_sb")
        bk_sb = const.tile([P, FC], f32, tag="bk_sb")

        # dithered bias copies for the ACT-side slot-1 evictions:
        # ACT computes Copy(ps*(1-h) + bq*(1-h)) == (ps + bq)*(1-h)
        bq_a = const.tile([P, FC], f32, tag="bq_a")
        bk_a = const.tile([P, FC], f32, tag="bk_a")
        bq_b = const.tile([P, FC], f32, tag="bq_b")
        bk_b = const.tile([P, FC], f32, tag="bk_b")

        iota_i = const.tile([P, KC], i32, tag="iota_i")
        nc.gpsimd.iota(iota_i[:], pattern=[[P, KC]], base=0, channel_multiplier=1)
        iota_f = const.tile([P, KC], f32, tag="iota_f")
        nc.gpsimd.tensor_copy(iota_f[:], iota_i[:])

        near_bc = const.tile([P, S], f32, tag="near_bc")
        nc.gpsimd.partition_broadcast(near_bc[:], near_row[:])
        bv_bc = const.tile([P, F], f32, tag="bv_bc")
        nc.gpsimd.partition_broadcast(bv_bc[:], bv_row[:])

        # q/k live as fp8e4 PAIRS: slot 0 holds fp8(v*(1+2^-4)), slot 1 holds
        # fp8(v*(1-2^-4)) — a half-quantization-cell dither. The DoubleRow
        # score matmul (0.5 cycles/row, half the bf16 PE cost) sums both
        # slots, averaging two anti-phased quantization errors (~2x less fp8
        # noise); the exact deterministic scale (a^2+b^2) plus 1/sqrt(d)
        # folds into the exp's scale operand. q itself is NOT pre-scaled so
        # fp8 sees healthy magnitudes.
        fp8 = mybir.dt.float8e4
        DIT = 1.0 / 16.0
        DSC = (1.0 + DIT) ** 2 + (1.0 - DIT) ** 2
        qT = big.tile([P, FC, 2, S], fp8, tag="qT")
        kT = big.tile([P, FC, 2, S], fp8, tag="kT")
        v_sb = big.tile([P, TC, HPC, 65], bf16, tag="v_sb")
        ones64 = const.tile([P, TC * HPC], f32, tag="ones64")
        nc.vector.memset(ones64[:], 1.0)
        nc.vector.tensor_copy(
            v_sb[:, :, :, 64:65],
            ones64.rearrange("p (a b one) -> p a b one", a=TC, b=HPC, one=1),
        )

        # warm-up matmuls on resident constants: the tensor engine p-state
        # ramps only under continuous execution (LOW->MID->full over ~3us),
        # so burn the ramp on throwaway [128x64 @ 128x32] products while the
        # first wv/x DMAs are still in flight — real matmuls then start at
        # full clock
        wps = ps_q.tile([P, 512], f32, tag="ps_proj", name="warmup")
        for w in range(28):
            nc.tensor.matmul(
                wps[0:64, 0:32],
                ones64[:, 0:64],
                ones64[:, 0:32],
                start=True, stop=True,
            )

        # small consts first on HWDGE (tiny transfers, needed by the earliest
        # DVE work): cb for the first exp, bq/bk for the q/k evictions
        nc.sync.dma_start(cb_sb[:], cb_d.rearrange("(jc p) -> p jc", p=P))
        nc.sync.dma_start(bq_sb[:], bqs_d.rearrange("(fc p) -> p fc", p=P))
        nc.sync.dma_start(bk_sb[:], bk_d.rearrange("(fc p) -> p fc", p=P))
        # dithered bias copies on DVE (idle until the first eviction) so they
        # never queue behind anything slow
        nc.vector.tensor_scalar(
            bq_a[:], bq_sb[:], 1.0 + DIT, None, mybir.AluOpType.mult
        )
        nc.vector.tensor_scalar(
            bk_a[:], bk_sb[:], 1.0 + DIT, None, mybir.AluOpType.mult
        )
        nc.vector.tensor_scalar(
            bq_b[:], bq_sb[:], 1.0 - DIT, None, mybir.AluOpType.mult
        )
        nc.vector.tensor_scalar(
            bk_b[:], bk_sb[:], 1.0 - DIT, None, mybir.AluOpType.mult
        )

        # identity for the PE o-transposes at group ends
        ident = const.tile([P, P], bf16, tag="ident")
        iota_r = const.tile([P, P], i32, tag="iota_r")
        nc.gpsimd.iota(iota_r[:], pattern=[[1, P]], base=0, channel_multiplier=0)
        iota_rf = const.tile([P, P], f32, tag="iota_rf")
        nc.gpsimd.tensor_copy(iota_rf[:], iota_r[:])
        iota_c = const.tile([P, 1], i32, tag="iota_c")
        nc.gpsimd.iota(iota_c[:], pattern=[[1, 1]], base=0, channel_multiplier=1)
        iota_cf = const.tile([P, 1], f32, tag="iota_cf")
        nc.gpsimd.tensor_copy(iota_cf[:], iota_c[:])
        nc.gpsimd.tensor_scalar(
            ident[:], iota_rf[:], iota_cf[:], None, mybir.AluOpType.is_equal
        )

        # verb factor (transposed): ebT[p, jc, i] = exp(2 * (jc*128+p == nearest[i]))
        # pass 1 here on the otherwise-idle Pool: is_equal -> {0,1} bf16.
        # pass 2 (a DVE 4x affine (e^2-1)*x + 1 -> {1, e^2}) is emitted
        # lazily inside the first group's slots: emitting it here would
        # head-of-line-block the DVE queue's eviction work behind Pool
        EM1 = float(np.exp(2.0) - 1.0)
        ebT = big.tile([P, TC, S], bf16, tag="ebT")
        ohstT = big.tile([P, TC, S], bf16, tag="ohstT")
        for jc in range(TC):
            nc.gpsimd.tensor_scalar(
                ohstT[:, jc, :], near_bc[:], iota_f[:, jc : jc + 1], None,
                mybir.AluOpType.is_equal,
            )

        def emit_ebt_pass2(jc):
            nc.vector.tensor_scalar(
                ebT[:, jc, :], ohstT[:, jc, :], EM1, 1.0,
                mybir.AluOpType.mult, mybir.AluOpType.add,
            )

        projpool = ctx.enter_context(tc.tile_pool(name="projpool", bufs=1))
        wq_sb = projpool.tile([P, KC, F], fp8, tag="wq_sb")
        wqr_sb = projpool.tile([P, KC, F], fp8, tag="wqr_sb")
        wk_sb = projpool.tile([P, KC, F], fp8, tag="wk_sb")
        wkr_sb = projpool.tile([P, KC, F], fp8, tag="wkr_sb")
        xTh = []
        xTr = []
        for i in range(IC):
            xthalf = projpool.tile([P, KC, 512], fp8, tag=f"xT{i}", name=f"xT{i}")
            xTh.append(xthalf)
            xrhalf = projpool.tile([P, KC, 512], fp8, tag=f"xR{i}", name=f"xR{i}")
            xTr.append(xrhalf)

        attn2 = ctx.enter_context(tc.tile_pool(name="attn2", bufs=1))
        oT = attn2.tile([P, FC, S], bf16, tag="oT")
        wo_sb = attn2.tile([P, FC, H], bf16, tag="wo_sb")

        # ---- x arrives host-pre-transposed as fp8 hi+residual; projections
        # run as split-fp8 DoubleRow kc-pair chains: x8*w8 + xr*w8 + x8*wr
        # (the dropped xr*wr term is ~quantization-squared). Weights are
        # host-pre-scaled by 32 into fp8's normal range; 1/32 folds into the
        # q/k eviction scales and (for v) into host-side Wo/32 + bv*32 ----
        wv_sb = projpool.tile([P, KC, F], fp8, tag="wv_sb")
        wvr_sb = projpool.tile([P, KC, F], fp8, tag="wvr_sb")

        def emit_vproj(tci_range):
            for tci in tci_range:
                ps = ps_q.tile([P, 512], f32, tag="ps_proj")
                half = tci // 4
                tsl = slice((tci % 4) * P, (tci % 4 + 1) * P)
                k = 0
                # term-major: the wv-only terms run before wvr arrives
                for xs, ws in (
                    (xTh[half], wv_sb),
                    (xTr[half], wv_sb),
                    (xTh[half], wvr_sb),
                ):
                    for kc2 in range(0, KC, 2):
                        nc.tensor.matmul(
                            ps[:],
                            xs[:, kc2 : kc2 + 2, tsl],
                            ws[:, kc2 : kc2 + 2, :],
                            start=(k == 0), stop=(k == 11),
                            perf_mode=mybir.MatmulPerfMode.DoubleRow,
                        )
                        k += 1
                nc.vector.tensor_tensor(
                    v_sb[:, tci, :, 0:64],
                    ps.rearrange("p (h d) -> p h d", d=64),
                    bv_bc.rearrange("p (h d) -> p h d", d=64),
                    mybir.AluOpType.add,
                )

        # DMA chain ordered for earliest attention-stream start: q weights +
        # x half0 (first q/k chains ~7us in), v weights (the v projection
        # fills the kproj DMA wait), k weights, x half1, wo last
        nc.sync.dma_start(wq_sb[:], wq_d.rearrange("(kc p) f -> p kc f", p=P))
        nc.sync.dma_start(wqr_sb[:], wqr_d.rearrange("(kc p) f -> p kc f", p=P))
        nc.sync.dma_start(
            xTh[0][:], x_d[:, 0:512].rearrange("(kc p) s -> p kc s", p=P)
        )
        nc.sync.dma_start(
            xTr[0][:], xr_d[:, 0:512].rearrange("(kc p) s -> p kc s", p=P)
        )
        # v weights BEFORE the k weights: the whole v projection then fills
        # the PE while kproj's inputs are still in flight, so once the
        # scores/exp stream starts nothing DMA-gated sits in front of it
        nc.sync.dma_start(wv_sb[:], wv_d.rearrange("(kc p) f -> p kc f", p=P))
        nc.sync.dma_start(wvr_sb[:], wvr_d.rearrange("(kc p) f -> p kc f", p=P))
        nc.sync.dma_start(wk_sb[:], wk_d.rearrange("(kc p) f -> p kc f", p=P))
        nc.sync.dma_start(wkr_sb[:], wkr_d.rearrange("(kc p) f -> p kc f", p=P))
        nc.sync.dma_start(
            xTh[1][:], x_d[:, 512:1024].rearrange("(kc p) s -> p kc s", p=P)
        )
        nc.sync.dma_start(
            xTr[1][:], xr_d[:, 512:1024].rearrange("(kc p) s -> p kc s", p=P)
        )
        nc.sync.dma_start(wo_sb[:], wo_d.rearrange("(fc p) o -> p fc o", p=P))
        # preload the exp table while the DMA stream runs so the first real
        # activation doesn't eat the 1.3us LoadActFuncSet
        exp_warm = const.tile([1, 1], f32, tag="exp_warm")
        nc.scalar.activation(
            exp_warm[:], ones64[0:1, 0:1], mybir.ActivationFunctionType.Exp
        )
        # p-state keepers: junk DoubleRow matmuls gated on each weight tile's
        # DMA keep the tensor engine's clock ramping through the load phase
        for src in (wq_sb, wqr_sb):
            for _ in range(6):
                wps2 = ps_q.tile([P, 512], f32, tag="ps_proj", name="warm2")
                nc.tensor.matmul(
                    wps2[:],
                    src[:, 0:2, 0:128],
                    src[:, 0:2, 0:512],
                    start=True, stop=True,
                    perf_mode=mybir.MatmulPerfMode.DoubleRow,
                )

        # ---- attention interleaved with q projection, per head pair.
        # Both oc halves of a tci land in one zt tile -> one DMA per z row
        # block (halves the HWDGE/sem slots; the tail is store-latency bound)
        zts = {}

        def emit_oproj(tiles, tail=False):
            for n, (tci, oc) in enumerate(tiles):
                if tail and n % 2 == 1:
                    pszw = ps_s.tile([P, 1024], f32, tag="pssb", name="pszw")
                    psz = pszw[:, 0:512]
                else:
                    psz = ps_q.tile([P, 512], f32, tag="ps_proj")
                for fc in range(FC):
                    nc.tensor.matmul(
                        psz[:],
                        oT[:, fc, tci * P : (tci + 1) * P],
                        wo_sb[:, fc, oc * 512 : (oc + 1) * 512],
                        start=(fc == 0), stop=(fc == FC - 1),
                    )
                if tci not in zts:
                    zts[tci] = zpool.tile([P, H], bf16, tag="zt", name="zt")
                zt = zts[tci]
                # gpsimd cannot read PSUM on real HW — evictions go DVE/ACT
                if tail:
                    nc.scalar.copy(zt[:, oc * 512 : (oc + 1) * 512], psz[:])
                else:
                    nc.vector.tensor_copy(zt[:, oc * 512 : (oc + 1) * 512], psz[:])
                # last two row blocks ship per-oc half-stores so the final
                # DMA after the last eviction is half-sized
                split_store = tail and tci >= 6
                if split_store:
                    nc.sync.dma_start(
                        z_d[tci * P : (tci + 1) * P, oc * 512 : (oc + 1) * 512],
                        zt[:, oc * 512 : (oc + 1) * 512],
                    )
                    if oc == IC - 1:
                        del zts[tci]
                elif oc == IC - 1:
                    nc.sync.dma_start(z_d[tci * P : (tci + 1) * P, :], zt[:])
                    del zts[tci]

        # group-end epilogue, stage 1 (DVE): reciprocal of the denominator
        # column + one broadcast multiply -> o_div [tok, (isub, side, feat)]
        def emit_division(pso, fc4_, ic_):
            rl = rlpool.tile([P, 4, 2, 1], f32, tag="rl")
            nc.vector.reciprocal(rl[:], pso[:, :, :, 64:65])
            o_div = odpool.tile([P, 4, 2, 64], bf16, tag="o_div")
            nc.vector.tensor_tensor(
                o_div[:], pso[:, :, :, 0:64],
                rl.to_broadcast((P, 4, 2, 64)),
                mybir.AluOpType.mult,
            )
            return (o_div, fc4_, ic_)

        # stage 2 (PE + ACT): 4 transposes [tok, (side,feat)] -> [(side,feat),
        # tok] through a ps_proj psum slot, then one eviction into oT
        def flush_transposes(pend_t):
            for o_div, fc4_, ic_ in pend_t:
                tp = ps_q.tile([P, 512], bf16, tag="ps_proj", name="tp")
                for isub in range(4):
                    nc.tensor.transpose(
                        tp[:, isub * P : (isub + 1) * P],
                        o_div[:, isub, :, :],
                        ident[:],
                    )
                nc.scalar.copy(oT[:, fc4_, ic_ * 512 : (ic_ + 1) * 512], tp[:])
            pend_t.clear()

        def emit_qproj(fc, icq):
            ps = ps_q.tile([P, 512], f32, tag="ps_proj")
            fsl = slice(fc * P, (fc + 1) * P)
            k = 0
            for ws, xs in (
                (wq_sb, xTh[icq]),
                (wq_sb, xTr[icq]),
                (wqr_sb, xTh[icq]),
            ):
                for kc2 in range(0, KC, 2):
                    nc.tensor.matmul(
                        ps[:],
                        ws[:, kc2 : kc2 + 2, fsl],
                        xs[:, kc2 : kc2 + 2, :],
                        start=(k == 0), stop=(k == 11),
                        perf_mode=mybir.MatmulPerfMode.DoubleRow,
                    )
                    k += 1
            # psum holds 32*q (host-scaled weights); (1 +- DIT)/32 restores
            # scale and applies the dither in one tensor_scalar each
            nc.vector.tensor_scalar(
                qT[:, fc, 0, icq * 512 : (icq + 1) * 512], ps[:],
                (1.0 + DIT) / 32.0, bq_a[:, fc : fc + 1],
                mybir.AluOpType.mult, mybir.AluOpType.add,
            )
            nc.vector.tensor_scalar(
                qT[:, fc, 1, icq * 512 : (icq + 1) * 512], ps[:],
                (1.0 - DIT) / 32.0, bq_b[:, fc : fc + 1],
                mybir.AluOpType.mult, mybir.AluOpType.add,
            )

        def emit_kproj(fc, ick):
            ps = ps_q.tile([P, 512], f32, tag="ps_proj")
            fsl = slice(fc * P, (fc + 1) * P)
            k = 0
            for ws, xs in (
                (wk_sb, xTh[ick]),
                (wk_sb, xTr[ick]),
                (wkr_sb, xTh[ick]),
            ):
                for kc2 in range(0, KC, 2):
                    nc.tensor.matmul(
                        ps[:],
                        ws[:, kc2 : kc2 + 2, fsl],
                        xs[:, kc2 : kc2 + 2, :],
                        start=(k == 0), stop=(k == 11),
                        perf_mode=mybir.MatmulPerfMode.DoubleRow,
                    )
                    k += 1
            nc.vector.tensor_scalar(
                kT[:, fc, 0, ick * 512 : (ick + 1) * 512], ps[:],
                (1.0 + DIT) / 32.0, bk_a[:, fc : fc + 1],
                mybir.AluOpType.mult, mybir.AluOpType.add,
            )
            nc.scalar.activation(
                kT[:, fc, 1, ick * 512 : (ick + 1) * 512], ps[:],
                mybir.ActivationFunctionType.Identity,
                bias=bk_b[:, fc : fc + 1], scale=(1.0 - DIT) / 32.0,
            )

        pend_t = []
        oproj_ic0 = [(tci, oc) for tci in range(4) for oc in range(IC)]
        for fc4 in range(FC):
            if fc4 == 0:
                emit_qproj(0, 0)
                # the full v projection sits between qproj and kproj: its wv
                # inputs land before the k weights, so it fills the PE during
                # the kproj DMA wait and clears all v deps before the stream
                emit_vproj(range(0, 8))
                emit_kproj(0, 0)
            for ic in range(IC):
                # heads of the pair interleaved: PE alternates A/B matmuls
                # while ACT/DVE process the other head's exp / verb multiply
                last_group = (fc4 == FC - 1 and ic == IC - 1)
                # flipped attn@v: out [tok(i), 65] with p as the stationary
                # operand puts all 128 PE output partitions to work (the old
                # [65, tok] orientation used 65 of 128) and lands the softmax
                # denominator on the partition axis where the division is one
                # broadcast multiply. [P, 4(isub), 2(side), 128] keeps every
                # accumulation chunk 512B-aligned inside the 2 psum banks.
                pso = ps_o.tile([P, 4, 2, 128], f32, tag="pso", name="pso")
                def emit_attnv(jc, pTb):
                    for isub in range(4):
                        for side in range(2):
                            h = 2 * fc4 + side
                            # psum start=True resets the WHOLE bank: only the
                            # first chain per bank (isub 0/2, side 0) carries
                            # it; the reset zeroes the sibling regions so the
                            # other chains accumulate from there (start=False)
                            nc.tensor.matmul(
                                pso[:, isub, side, 0:65],
                                pTb[:, side * 512 + isub * P : side * 512 + (isub + 1) * P],
                                v_sb[:, jc, h, 0:65],
                                start=(jc == 0 and side == 0 and isub % 2 == 0),
                                stop=(jc == TC - 1),
                            )

                lag = 0
                pTbs = {}

                def emit_slot_top(jc):
                    # this jc's scores + exp first: the exp fires early in the
                    # slot so the ACT stream never waits on the fill work below
                    pssb = ps_s.tile([P, 1024], f32, tag="pssb")
                    for side in range(2):
                        hb = side * 64
                        nc.tensor.matmul(
                            pssb[:, side * 512 : (side + 1) * 512],
                            kT[hb : hb + 64, fc4, 0:2, jc * P : (jc + 1) * P],
                            qT[hb : hb + 64, fc4, 0:2, ic * 512 : (ic + 1) * 512],
                            start=True, stop=True,
                            perf_mode=mybir.MatmulPerfMode.DoubleRow,
                        )
                    pTb = ppool.tile([P, 1024], bf16, tag="pTb")
                    nc.scalar.activation(
                        pTb[:], pssb[:], mybir.ActivationFunctionType.Exp,
                        bias=cb_sb[:, jc : jc + 1], scale=SCALE / DSC,
                    )
                    if fc4 == 0 and ic == 0:
                        emit_ebt_pass2(jc)
                    ebsl = ebT[:, jc, ic * 512 : (ic + 1) * 512]
                    nc.vector.tensor_tensor(
                        pTb.rearrange("p (two n) -> p two n", two=2),
                        pTb.rearrange("p (two n) -> p two n", two=2),
                        ebsl[:, None, :].to_broadcast((P, 2, 512)),
                        mybir.AluOpType.mult,
                    )
                    pTbs[jc] = pTb

                for jc in range(TC):
                    emit_slot_top(jc)
                    # ---- slot fills: projection chains, transposes, o_proj
                    # drips — all behind the slot's scores/exp so the ACT
                    # cadence never blocks on them
                    if ic == 0 and jc == 2:
                        # second-half k projection rides inside the i0 group
                        # (term-major: for fc0 it starts on x half1 arrival,
                        # just ahead of the jc4 scores that need its output)
                        emit_kproj(fc4, 1)
                    if ic == 1 and fc4 < FC - 1:
                        # next head-pair's first-half projections fill the
                        # otherwise-bare i1 groups (needed a full group later)
                        if jc == 2:
                            emit_qproj(fc4 + 1, 0)
                        if jc == 5:
                            emit_kproj(fc4 + 1, 0)
                    if jc == 2 and pend_t:
                        # previous group's o_div is ready by now (its division
                        # ran on DVE at the group boundary) — the transposes
                        # slot into the PE stream without a sem stall
                        flush_transposes(pend_t)
                    if last_group and jc >= 3 and oproj_ic0:
                        # (f3,i0)'s oT lands via the jc2 transpose flush just
                        # above; drip its o_proj tiles through the final group
                        emit_oproj(oproj_ic0[:1])
                        del oproj_ic0[:1]
                    # same-slot attn@v at the bottom: by now the DVE verb
                    # multiply for this jc is done, so it issues cleanly and
                    # the next slot's scores are never held up
                    if lag == 0:
                        emit_attnv(jc, pTbs.pop(jc))
                for jc in range(TC - lag, TC):
                    emit_attnv(jc, pTbs.pop(jc))
                if ic == 0:
                    # boundary chain: keeps the PE busy across the i0->i1
                    # group switch while DVE drains the evictions
                    emit_qproj(fc4, 1)
                pend_t.append(emit_division(pso, fc4, ic))
        flush_transposes(pend_t)
        if oproj_ic0:
            emit_oproj(oproj_ic0)
        emit_oproj([(tci, oc) for tci in range(4, 8) for oc in range(IC)], tail=True)

    nc.compile()
    return nc


def _get_compiled():
    global _COMPILED
    if _COMPILED is None:
        _COMPILED = _build()
    return _COMPILED


def _host_morpho(morpho_types):
    """nearest-verb index per (b, i) (-1 if batch has no verb) and col bias."""
    mt = np.asarray(morpho_types)
    pos = np.arange(S)
    dist = np.abs(pos[:, None] - pos[None, :]).astype(np.float32)
    nearest = np.empty((B, S), np.float32)
    for b in range(B):
        is_verb = mt[b] == 2
        if not is_verb.any():
            nearest[b] = -1.0
            continue
        dm = np.where(is_verb[None, :], dist, BIG)
        nearest[b] = np.argmin(dm, axis=-1).astype(np.float32)
    cb = (
        np.float32(ROOT_BIAS * 0.5) * (mt == 0)
        + np.float32(SUFFIX_BIAS * 0.3) * (mt == 1)
    ).astype(np.float32)
    return nearest, cb


def _fp8_split(a):
    f8 = ml_dtypes.float8_e4m3
    hi = np.ascontiguousarray(a.astype(f8))
    res = np.ascontiguousarray((a - hi.astype(np.float32)).astype(f8))
    return hi, res


def build_in_maps(hidden_states, morpho_types, Wq, bq, Wk, bk, Wv, bv, Wo, bo):
    # weights are pre-scaled by 32 into fp8's normal range (their raw 0.02
    # scale sits in e4m3 subnormals); 1/32 is folded into the q/k eviction
    # scales and, for the v path, into Wo/32 with bv*32 (the softmax
    # denominator is v-scale-invariant)
    hidden_states = np.ascontiguousarray(np.asarray(hidden_states, np.float32))
    bft = ml_dtypes.bfloat16
    Wq = np.asarray(Wq, np.float32) * np.float32(32.0)
    Wk = np.asarray(Wk, np.float32) * np.float32(32.0)
    Wv = np.asarray(Wv, np.float32) * np.float32(32.0)
    Wo = (np.asarray(Wo, np.float32) / np.float32(32.0)).astype(bft)
    bq = np.asarray(bq, np.float32)
    bk = np.asarray(bk, np.float32)
    bv = np.asarray(bv, np.float32) * np.float32(32.0)

    nearest, cb = _host_morpho(morpho_types)

    in_maps = []
    for c in range(8):
        b, g = c // G, c % G
        fs = slice(g * F, (g + 1) * F)
        x8, xr8 = _fp8_split(hidden_states[b].T)
        wq8, wqr8 = _fp8_split(Wq[:, fs])
        wk8, wkr8 = _fp8_split(Wk[:, fs])
        wv8, wvr8 = _fp8_split(Wv[:, fs])
        in_maps.append({
            "x": x8, "xr": xr8,
            "wq": wq8, "wqr": wqr8,
            "wk": wk8, "wkr": wkr8,
            "wv": wv8, "wvr": wvr8,
            "wo": np.ascontiguousarray(Wo[fs, :]),
            "bqs": np.ascontiguousarray(bq[fs]),
            "bk": np.ascontiguousarray(bk[fs]),
            "bv": np.ascontiguousarray(bv[fs]),
            "nearf": nearest[b],
            "cb": cb[b],
        })
    return in_maps


def kernel(hidden_states, morpho_types, Wq, bq, Wk, bk, Wv, bv, Wo, bo):
    bo = np.asarray(bo, np.float32)
    in_maps = build_in_maps(
        hidden_states, morpho_types, Wq, bq, Wk, bk, Wv, bv, Wo, bo
    )
    nc = _get_compiled()
    res = run_bass_kernel_spmd(nc, in_maps, core_ids=list(range(8)))
    out = np.empty((B, S, H), np.float32)
    for b in range(B):
        out[b] = (
            res.results[2 * b]["z"].astype(np.float32)
            + res.results[2 * b + 1]["z"].astype(np.float32)
            + bo
        )
    return out



# revision 41
# speedup vs baseline: 1.0228x; 1.0228x over previous
"""AgglutinativeAttention Trainium2 kernel.

Full inputs in, full output out. Sharding: 8 cores = (batch b in 0..3) x
(head-group g in 0..1). Each core computes, for its batch b and its 8 heads:
  qT/kT = (x @ W{q,k}[:, gF:(g+1)F])^T   [512 feat, 1024 tok]
  v     =  x @ Wv[:, gF:(g+1)F]          [1024 tok, 512 feat] (+ones col/head)
  per head: sT = scores transposed [j, i] via fp8 DoubleRow matmuls (dithered
  dual quantization, see the qT/kT comment), morpho verb bias via an ebT
  elementwise factor + per-partition activation bias (col bias),
  pT = exp(scale*sT + cb) * ebT, oT = v_aug^T @ pT with a ones row giving the
  softmax denominator, divide, then partial z = o @ Wo[gF:(g+1)F, :].
  Host sums the two per-batch partials + bo.

x and the q/k/v weights travel as fp8 hi+residual pairs (split-fp8
DoubleRow projections at 0.75x the bf16 PE cost), wo and z as bf16. DMA
order is arranged so the first v-proj matmul is gated on a few hundred KB,
not 7MB; warm-up matmuls burn the PE p-state ramp during the initial DMA
wait.
"""

import numpy as np
import ml_dtypes
from contextlib import ExitStack

import concourse.bass as bass
import concourse.mybir as mybir
import concourse.tile as tile
from concourse import bacc
from concourse.bass_utils import run_bass_kernel_spmd

B, S, H = 4, 1024, 1024
NH, HD = 16, 64
G = 2                 # head groups (tensor-parallel factor per batch)
F = H // G            # 512 features per core
HPC = NH // G         # 8 heads per core
SCALE = 1.0 / np.sqrt(HD)
VERB_BIAS, ROOT_BIAS, SUFFIX_BIAS = 2.0, 1.5, 1.2
BIG = np.float32(1e9)

f32 = mybir.dt.float32
f32r = mybir.dt.float32r
bf16 = mybir.dt.bfloat16
i32 = mybir.dt.int32

P = 128
KC = H // P           # 8 contraction chunks for projections
TC = S // P           # 8 token chunks of 128
IC = S // 512         # 2 chunks of 512 (matmul free dim)
FC = F // P           # 4 feature chunks per core

_COMPILED = None


def _build():
    nc = bacc.Bacc("TRN2", target_bir_lowering=False, debug=False, num_devices=8)

    fp8d = mybir.dt.float8e4
    x_d = nc.dram_tensor("x", [H, S], fp8d, kind="ExternalInput").ap()
    xr_d = nc.dram_tensor("xr", [H, S], fp8d, kind="ExternalInput").ap()
    wq_d = nc.dram_tensor("wq", [H, F], fp8d, kind="ExternalInput").ap()
    wqr_d = nc.dram_tensor("wqr", [H, F], fp8d, kind="ExternalInput").ap()
    wk_d = nc.dram_tensor("wk", [H, F], fp8d, kind="ExternalInput").ap()
    wkr_d = nc.dram_tensor("wkr", [H, F], fp8d, kind="ExternalInput").ap()
    wv_d = nc.dram_tensor("wv", [H, F], fp8d, kind="ExternalInput").ap()
    wvr_d = nc.dram_tensor("wvr", [H, F], fp8d, kind="ExternalInput").ap()
    wo_d = nc.dram_tensor("wo", [F, H], bf16, kind="ExternalInput").ap()
    # cb [P, TC] | bq [P, FC] | bk [P, FC] packed host-side into one small
    # transfer: three separate HWDGE generations would serialize ~1.9us ahead
    # of the big weight transfers
    consts_d = nc.dram_tensor("consts", [P, TC + 2 * FC], f32, kind="ExternalInput").ap()
    bv_d = nc.dram_tensor("bv", [F], f32, kind="ExternalInput").ap()
    nearf_d = nc.dram_tensor("nearf", [S], f32, kind="ExternalInput").ap()
    z_d = nc.dram_tensor("z", [S, H], bf16, kind="ExternalOutput").ap()

    with tile.TileContext(nc) as tc, ExitStack() as ctx:
        const = ctx.enter_context(tc.tile_pool(name="const", bufs=1))
        big = ctx.enter_context(tc.tile_pool(name="big", bufs=1))
        ppool = ctx.enter_context(tc.tile_pool(name="ppool", bufs=6))
        rlpool = ctx.enter_context(tc.tile_pool(name="rlpool", bufs=2))
        odpool = ctx.enter_context(tc.tile_pool(name="odpool", bufs=2))
        zpool = ctx.enter_context(tc.tile_pool(name="zpool", bufs=6))
        ps_q = ctx.enter_context(tc.tile_pool(name="ps_q", bufs=2, space="PSUM"))
        ps_s = ctx.enter_context(tc.tile_pool(name="ps_s", bufs=2, space="PSUM"))
        ps_o = ctx.enter_context(tc.tile_pool(name="ps_o", bufs=1, space="PSUM"))

        # ---- small constants via SWDGE (gpsimd) — keeps HWDGE free for the
        # wv/x stream that gates the first matmul. Only the two needed early
        # (bv for the first v eviction, nearf for the ohst chain) go first;
        # the rest are emitted after the ohst loop so their transfers don't
        # sit ahead of the wv/x stream on the DMA engines ----
        near_row = const.tile([1, S], f32, tag="near_row")
        bv_row = const.tile([1, F], f32, tag="bv_row")
        nc.gpsimd.dma_start(bv_row[:], bv_d[None, :])
        nc.gpsimd.dma_start(near_row[:], nearf_d[None, :])
        consts_sb = const.tile([P, TC + 2 * FC], f32, tag="consts_sb")
        cb_sb = consts_sb[:, 0:TC]
        bq_sb = consts_sb[:, TC : TC + FC]
        bk_sb = consts_sb[:, TC + FC : TC + 2 * FC]

        # dithered bias copies for the ACT-side slot-1 evictions:
        # ACT computes Copy(ps*(1-h) + bq*(1-h)) == (ps + bq)*(1-h)
        bq_a = const.tile([P, FC], f32, tag="bq_a")
        bk_a = const.tile([P, FC], f32, tag="bk_a")
        bq_b = const.tile([P, FC], f32, tag="bq_b")
        bk_b = const.tile([P, FC], f32, tag="bk_b")

        iota_i = const.tile([P, KC], i32, tag="iota_i")
        nc.gpsimd.iota(iota_i[:], pattern=[[P, KC]], base=0, channel_multiplier=1)
        iota_f = const.tile([P, KC], f32, tag="iota_f")
        nc.gpsimd.tensor_copy(iota_f[:], iota_i[:])

        near_bc = const.tile([P, S], f32, tag="near_bc")
        nc.gpsimd.partition_broadcast(near_bc[:], near_row[:])
        bv_bc = const.tile([P, F], f32, tag="bv_bc")
        nc.gpsimd.partition_broadcast(bv_bc[:], bv_row[:])

        # q/k live as fp8e4 PAIRS: slot 0 holds fp8(v*(1+2^-4)), slot 1 holds
        # fp8(v*(1-2^-4)) — a half-quantization-cell dither. The DoubleRow
        # score matmul (0.5 cycles/row, half the bf16 PE cost) sums both
        # slots, averaging two anti-phased quantization errors (~2x less fp8
        # noise); the exact deterministic scale (a^2+b^2) plus 1/sqrt(d)
        # folds into the exp's scale operand. q itself is NOT pre-scaled so
        # fp8 sees healthy magnitudes.
        fp8 = mybir.dt.float8e4
        DIT = 1.0 / 16.0
        DSC = (1.0 + DIT) ** 2 + (1.0 - DIT) ** 2
        qT = big.tile([P, FC, 2, S], fp8, tag="qT")
        kT = big.tile([P, FC, 2, S], fp8, tag="kT")
        v_sb = big.tile([P, TC, HPC, 65], bf16, tag="v_sb")
        ones64 = const.tile([P, TC * HPC], f32, tag="ones64")
        nc.vector.memset(ones64[:], 1.0)
        nc.vector.tensor_copy(
            v_sb[:, :, :, 64:65],
            ones64.rearrange("p (a b one) -> p a b one", a=TC, b=HPC, one=1),
        )

        # warm-up matmuls on resident constants: the tensor engine p-state
        # ramps only under continuous execution (LOW->MID->full over ~3us),
        # so burn the ramp on throwaway [128x64 @ 128x32] products while the
        # first wv/x DMAs are still in flight — real matmuls then start at
        # full clock
        wps = ps_q.tile([P, 512], f32, tag="ps_proj", name="warmup")
        for w in range(28):
            nc.tensor.matmul(
                wps[0:64, 0:32],
                ones64[:, 0:64],
                ones64[:, 0:32],
                start=True, stop=True,
            )

        # small consts first on HWDGE (one tiny transfer, needed by the
        # earliest DVE work): cb for the first exp, bq/bk for the evictions
        nc.sync.dma_start(consts_sb[:], consts_d)
        # dithered bias copies on DVE (idle until the first eviction) so they
        # never queue behind anything slow
        nc.vector.tensor_scalar(
            bq_a[:], bq_sb[:], 1.0 + DIT, None, mybir.AluOpType.mult
        )
        nc.vector.tensor_scalar(
            bk_a[:], bk_sb[:], 1.0 + DIT, None, mybir.AluOpType.mult
        )
        nc.vector.tensor_scalar(
            bq_b[:], bq_sb[:], 1.0 - DIT, None, mybir.AluOpType.mult
        )
        nc.vector.tensor_scalar(
            bk_b[:], bk_sb[:], 1.0 - DIT, None, mybir.AluOpType.mult
        )

        # identity for the PE o-transposes at group ends
        ident = const.tile([P, P], bf16, tag="ident")
        iota_r = const.tile([P, P], i32, tag="iota_r")
        nc.gpsimd.iota(iota_r[:], pattern=[[1, P]], base=0, channel_multiplier=0)
        iota_rf = const.tile([P, P], f32, tag="iota_rf")
        nc.gpsimd.tensor_copy(iota_rf[:], iota_r[:])
        iota_c = const.tile([P, 1], i32, tag="iota_c")
        nc.gpsimd.iota(iota_c[:], pattern=[[1, 1]], base=0, channel_multiplier=1)
        iota_cf = const.tile([P, 1], f32, tag="iota_cf")
        nc.gpsimd.tensor_copy(iota_cf[:], iota_c[:])
        nc.gpsimd.tensor_scalar(
            ident[:], iota_rf[:], iota_cf[:], None, mybir.AluOpType.is_equal
        )

        # verb factor (transposed): ebT[p, jc, i] = exp(2 * (jc*128+p == nearest[i]))
        # both passes on DVE, compact and early (DVE is otherwise idle before
        # the first eviction): is_equal -> {0,1} bf16, then a 4x tensor_scalar
        # affine (e^2-1)*x + 1 -> {1, e^2}. Putting pass 1 on Pool spreads
        # the chain over 17us of Pool pacing and starves the eviction work.
        EM1 = float(np.exp(2.0) - 1.0)
        ebT = big.tile([P, TC, S], bf16, tag="ebT")
        ohstage = ctx.enter_context(tc.tile_pool(name="ohstage", bufs=2))
        for jc in range(TC):
            ohst = ohstage.tile([P, S], bf16, tag="ohst")
            nc.vector.tensor_scalar(
                ohst[:], near_bc[:], iota_f[:, jc : jc + 1], None,
                mybir.AluOpType.is_equal,
            )
            nc.vector.tensor_scalar(
                ebT[:, jc, :], ohst[:], EM1, 1.0,
                mybir.AluOpType.mult, mybir.AluOpType.add,
            )

        projpool = ctx.enter_context(tc.tile_pool(name="projpool", bufs=1))
        wq_sb = projpool.tile([P, KC, F], fp8, tag="wq_sb")
        wqr_sb = projpool.tile([P, KC, F], fp8, tag="wqr_sb")
        wk_sb = projpool.tile([P, KC, F], fp8, tag="wk_sb")
        wkr_sb = projpool.tile([P, KC, F], fp8, tag="wkr_sb")
        xTh = []
        xTr = []
        for i in range(IC):
            xthalf = projpool.tile([P, KC, 512], fp8, tag=f"xT{i}", name=f"xT{i}")
            xTh.append(xthalf)
            xrhalf = projpool.tile([P, KC, 512], fp8, tag=f"xR{i}", name=f"xR{i}")
            xTr.append(xrhalf)

        attn2 = ctx.enter_context(tc.tile_pool(name="attn2", bufs=1))
        oT = attn2.tile([P, FC, S], bf16, tag="oT")
        wo_sb = attn2.tile([P, FC, H], bf16, tag="wo_sb")

        # ---- x arrives host-pre-transposed as fp8 hi+residual; projections
        # run as split-fp8 DoubleRow kc-pair chains: x8*w8 + xr*w8 + x8*wr
        # (the dropped xr*wr term is ~quantization-squared). Weights are
        # host-pre-scaled by 32 into fp8's normal range; 1/32 folds into the
        # q/k eviction scales and (for v) into host-side Wo/32 + bv*32 ----
        wv_sb = projpool.tile([P, KC, F], fp8, tag="wv_sb")
        wvr_sb = projpool.tile([P, KC, F], fp8, tag="wvr_sb")

        def emit_vproj(tci_range):
            for tci in tci_range:
                ps = ps_q.tile([P, 512], f32, tag="ps_proj")
                half = tci // 4
                tsl = slice((tci % 4) * P, (tci % 4 + 1) * P)
                k = 0
                # term-major: the wv-only terms run before wvr arrives
                for xs, ws in (
                    (xTh[half], wv_sb),
                    (xTr[half], wv_sb),
                    (xTh[half], wvr_sb),
                ):
                    for kc2 in range(0, KC, 2):
                        nc.tensor.matmul(
                            ps[:],
                            xs[:, kc2 : kc2 + 2, tsl],
                            ws[:, kc2 : kc2 + 2, :],
                            start=(k == 0), stop=(k == 11),
                            perf_mode=mybir.MatmulPerfMode.DoubleRow,
                        )
                        k += 1
                nc.vector.tensor_tensor(
                    v_sb[:, tci, :, 0:64],
                    ps.rearrange("p (h d) -> p h d", d=64),
                    bv_bc.rearrange("p (h d) -> p h d", d=64),
                    mybir.AluOpType.add,
                )

        # DMA chain ordered for earliest attention-stream start: q weights +
        # x half0 (first q/k chains ~7us in), v weights (the v projection
        # fills the kproj DMA wait), k weights, x half1, wo last
        nc.sync.dma_start(wq_sb[:], wq_d.rearrange("(kc p) f -> p kc f", p=P))
        nc.sync.dma_start(wqr_sb[:], wqr_d.rearrange("(kc p) f -> p kc f", p=P))
        nc.sync.dma_start(
            xTh[0][:], x_d[:, 0:512].rearrange("(kc p) s -> p kc s", p=P)
        )
        nc.sync.dma_start(
            xTr[0][:], xr_d[:, 0:512].rearrange("(kc p) s -> p kc s", p=P)
        )
        # v weights BEFORE the k weights: the whole v projection then fills
        # the PE while kproj's inputs are still in flight, so once the
        # scores/exp stream starts nothing DMA-gated sits in front of it
        nc.sync.dma_start(wv_sb[:], wv_d.rearrange("(kc p) f -> p kc f", p=P))
        nc.sync.dma_start(wvr_sb[:], wvr_d.rearrange("(kc p) f -> p kc f", p=P))
        nc.sync.dma_start(wk_sb[:], wk_d.rearrange("(kc p) f -> p kc f", p=P))
        nc.sync.dma_start(wkr_sb[:], wkr_d.rearrange("(kc p) f -> p kc f", p=P))
        nc.sync.dma_start(
            xTh[1][:], x_d[:, 512:1024].rearrange("(kc p) s -> p kc s", p=P)
        )
        nc.sync.dma_start(
            xTr[1][:], xr_d[:, 512:1024].rearrange("(kc p) s -> p kc s", p=P)
        )
        nc.sync.dma_start(wo_sb[:], wo_d.rearrange("(fc p) o -> p fc o", p=P))
        # preload the exp table while the DMA stream runs so the first real
        # activation doesn't eat the 1.3us LoadActFuncSet
        exp_warm = const.tile([1, 1], f32, tag="exp_warm")
        nc.scalar.activation(
            exp_warm[:], ones64[0:1, 0:1], mybir.ActivationFunctionType.Exp
        )
        # p-state keepers: junk DoubleRow matmuls gated on each weight tile's
        # DMA keep the tensor engine's clock ramping through the load phase
        for src in (wq_sb, wqr_sb):
            for _ in range(6):
                wps2 = ps_q.tile([P, 512], f32, tag="ps_proj", name="warm2")
                nc.tensor.matmul(
                    wps2[:],
                    src[:, 0:2, 0:128],
                    src[:, 0:2, 0:512],
                    start=True, stop=True,
                    perf_mode=mybir.MatmulPerfMode.DoubleRow,
                )

        # ---- attention interleaved with q projection, per head pair.
        # Both oc halves of a tci land in one zt tile -> one DMA per z row
        # block (halves the HWDGE/sem slots; the tail is store-latency bound)
        zts = {}

        def emit_oproj(tiles, tail=False):
            for n, (tci, oc) in enumerate(tiles):
                if tail and n % 2 == 1:
                    pszw = ps_s.tile([P, 1024], f32, tag="pssb", name="pszw")
                    psz = pszw[:, 0:512]
                else:
                    psz = ps_q.tile([P, 512], f32, tag="ps_proj")
                for fc in range(FC):
                    nc.tensor.matmul(
                        psz[:],
                        oT[:, fc, tci * P : (tci + 1) * P],
                        wo_sb[:, fc, oc * 512 : (oc + 1) * 512],
                        start=(fc == 0), stop=(fc == FC - 1),
                    )
                if tci not in zts:
                    zts[tci] = zpool.tile([P, H], bf16, tag="zt", name="zt")
                zt = zts[tci]
                # gpsimd cannot read PSUM on real HW — evictions go DVE/ACT
                if tail:
                    nc.scalar.copy(zt[:, oc * 512 : (oc + 1) * 512], psz[:])
                else:
                    nc.vector.tensor_copy(zt[:, oc * 512 : (oc + 1) * 512], psz[:])
                # last two row blocks ship per-oc half-stores so the final
                # DMA after the last eviction is half-sized
                split_store = tail and tci >= 6
                if split_store:
                    nc.sync.dma_start(
                        z_d[tci * P : (tci + 1) * P, oc * 512 : (oc + 1) * 512],
                        zt[:, oc * 512 : (oc + 1) * 512],
                    )
                    if oc == IC - 1:
                        del zts[tci]
                elif oc == IC - 1:
                    nc.sync.dma_start(z_d[tci * P : (tci + 1) * P, :], zt[:])
                    del zts[tci]

        # group-end epilogue, stage 1 (DVE): reciprocal of the denominator
        # column + one broadcast multiply -> o_div [tok, (isub, side, feat)]
        def emit_division(pso, fc4_, ic_):
            rl = rlpool.tile([P, 4, 2, 1], f32, tag="rl")
            nc.vector.reciprocal(rl[:], pso[:, :, :, 64:65])
            o_div = odpool.tile([P, 4, 2, 64], bf16, tag="o_div")
            nc.vector.tensor_tensor(
                o_div[:], pso[:, :, :, 0:64],
                rl.to_broadcast((P, 4, 2, 64)),
                mybir.AluOpType.mult,
            )
            return (o_div, fc4_, ic_)

        # stage 2 (PE + ACT): 4 transposes [tok, (side,feat)] -> [(side,feat),
        # tok] through a ps_proj psum slot, then one eviction into oT
        def flush_transposes(pend_t):
            for o_div, fc4_, ic_ in pend_t:
                tp = ps_q.tile([P, 512], bf16, tag="ps_proj", name="tp")
                for isub in range(4):
                    nc.tensor.transpose(
                        tp[:, isub * P : (isub + 1) * P],
                        o_div[:, isub, :, :],
                        ident[:],
                    )
                nc.scalar.copy(oT[:, fc4_, ic_ * 512 : (ic_ + 1) * 512], tp[:])
            pend_t.clear()

        def emit_qproj(fc, icq):
            ps = ps_q.tile([P, 512], f32, tag="ps_proj")
            fsl = slice(fc * P, (fc + 1) * P)
            k = 0
            for ws, xs in (
                (wq_sb, xTh[icq]),
                (wq_sb, xTr[icq]),
                (wqr_sb, xTh[icq]),
            ):
                for kc2 in range(0, KC, 2):
                    nc.tensor.matmul(
                        ps[:],
                        ws[:, kc2 : kc2 + 2, fsl],
                        xs[:, kc2 : kc2 + 2, :],
                        start=(k == 0), stop=(k == 11),
                        perf_mode=mybir.MatmulPerfMode.DoubleRow,
                    )
                    k += 1
            # psum holds 32*q (host-scaled weights); (1 +- DIT)/32 restores
            # scale and applies the dither in one tensor_scalar each
            nc.vector.tensor_scalar(
                qT[:, fc, 0, icq * 512 : (icq + 1) * 512], ps[:],
                (1.0 + DIT) / 32.0, bq_a[:, fc : fc + 1],
                mybir.AluOpType.mult, mybir.AluOpType.add,
            )
            nc.vector.tensor_scalar(
                qT[:, fc, 1, icq * 512 : (icq + 1) * 512], ps[:],
                (1.0 - DIT) / 32.0, bq_b[:, fc : fc + 1],
                mybir.AluOpType.mult, mybir.AluOpType.add,
            )

        def emit_kproj(fc, ick):
            ps = ps_q.tile([P, 512], f32, tag="ps_proj")
            fsl = slice(fc * P, (fc + 1) * P)
            k = 0
            for ws, xs in (
                (wk_sb, xTh[ick]),
                (wk_sb, xTr[ick]),
                (wkr_sb, xTh[ick]),
            ):
                for kc2 in range(0, KC, 2):
                    nc.tensor.matmul(
                        ps[:],
                        ws[:, kc2 : kc2 + 2, fsl],
                        xs[:, kc2 : kc2 + 2, :],
                        start=(k == 0), stop=(k == 11),
                        perf_mode=mybir.MatmulPerfMode.DoubleRow,
                    )
                    k += 1
            nc.vector.tensor_scalar(
                kT[:, fc, 0, ick * 512 : (ick + 1) * 512], ps[:],
                (1.0 + DIT) / 32.0, bk_a[:, fc : fc + 1],
                mybir.AluOpType.mult, mybir.AluOpType.add,
            )
            nc.scalar.activation(
                kT[:, fc, 1, ick * 512 : (ick + 1) * 512], ps[:],
                mybir.ActivationFunctionType.Identity,
                bias=bk_b[:, fc : fc + 1], scale=(1.0 - DIT) / 32.0,
            )

        pend_t = []
        oproj_ic0 = [(tci, oc) for tci in range(4) for oc in range(IC)]
        for fc4 in range(FC):
            if fc4 == 0:
                emit_qproj(0, 0)
                # the full v projection sits between qproj and kproj: its wv
                # inputs land before the k weights, so it fills the PE during
                # the kproj DMA wait and clears all v deps before the stream
                emit_vproj(range(0, 8))
                emit_kproj(0, 0)
            for ic in range(IC):
                # heads of the pair interleaved: PE alternates A/B matmuls
                # while ACT/DVE process the other head's exp / verb multiply
                last_group = (fc4 == FC - 1 and ic == IC - 1)
                # flipped attn@v: out [tok(i), 65] with p as the stationary
                # operand puts all 128 PE output partitions to work (the old
                # [65, tok] orientation used 65 of 128) and lands the softmax
                # denominator on the partition axis where the division is one
                # broadcast multiply. [P, 4(isub), 2(side), 128] keeps every
                # accumulation chunk 512B-aligned inside the 2 psum banks.
                pso = ps_o.tile([P, 4, 2, 128], f32, tag="pso", name="pso")
                def emit_attnv(jc, pTb):
                    for isub in range(4):
                        for side in range(2):
                            h = 2 * fc4 + side
                            # psum start=True resets the WHOLE bank: only the
                            # first chain per bank (isub 0/2, side 0) carries
                            # it; the reset zeroes the sibling regions so the
                            # other chains accumulate from there (start=False)
                            nc.tensor.matmul(
                                pso[:, isub, side, 0:65],
                                pTb[:, side * 512 + isub * P : side * 512 + (isub + 1) * P],
                                v_sb[:, jc, h, 0:65],
                                start=(jc == 0 and side == 0 and isub % 2 == 0),
                                stop=(jc == TC - 1),
                            )

                lag = 0
                pTbs = {}

                def emit_slot_top(jc):
                    # this jc's scores + exp first: the exp fires early in the
                    # slot so the ACT stream never waits on the fill work below
                    pssb = ps_s.tile([P, 1024], f32, tag="pssb")
                    for side in range(2):
                        hb = side * 64
                        nc.tensor.matmul(
                            pssb[:, side * 512 : (side + 1) * 512],
                            kT[hb : hb + 64, fc4, 0:2, jc * P : (jc + 1) * P],
                            qT[hb : hb + 64, fc4, 0:2, ic * 512 : (ic + 1) * 512],
                            start=True, stop=True,
                            perf_mode=mybir.MatmulPerfMode.DoubleRow,
                        )
                    pTb = ppool.tile([P, 1024], bf16, tag="pTb")
                    nc.scalar.activation(
                        pTb[:], pssb[:], mybir.ActivationFunctionType.Exp,
                        bias=cb_sb[:, jc : jc + 1], scale=SCALE / DSC,
                    )
                    ebsl = ebT[:, jc, ic * 512 : (ic + 1) * 512]
                    nc.vector.tensor_tensor(
                        pTb.rearrange("p (two n) -> p two n", two=2),
                        pTb.rearrange("p (two n) -> p two n", two=2),
                        ebsl[:, None, :].to_broadcast((P, 2, 512)),
                        mybir.AluOpType.mult,
                    )
                    pTbs[jc] = pTb

                for jc in range(TC):
                    emit_slot_top(jc)
                    # ---- slot fills: projection chains, transposes, o_proj
                    # drips — all behind the slot's scores/exp so the ACT
                    # cadence never blocks on them
                    if ic == 0 and jc == 2:
                        # second-half k projection rides inside the i0 group
                        # (term-major: for fc0 it starts on x half1 arrival,
                        # just ahead of the jc4 scores that need its output)
                        emit_kproj(fc4, 1)
                    if ic == 1 and fc4 < FC - 1:
                        # next head-pair's first-half projections fill the
                        # otherwise-bare i1 groups (needed a full group later)
                        if jc == 2:
                            emit_qproj(fc4 + 1, 0)
                        if jc == 5:
                            emit_kproj(fc4 + 1, 0)
                    if jc == 2 and pend_t:
                        # previous group's o_div is ready by now (its division
                        # ran on DVE at the group boundary) — the transposes
                        # slot into the PE stream without a sem stall
                        flush_transposes(pend_t)
                    if last_group and jc >= 3 and oproj_ic0:
                        # (f3,i0)'s oT lands via the jc2 transpose flush just
                        # above; drip its o_proj tiles through the final group
                        emit_oproj(oproj_ic0[:1])
                        del oproj_ic0[:1]
                    # same-slot attn@v at the bottom: by now the DVE verb
                    # multiply for this jc is done, so it issues cleanly and
                    # the next slot's scores are never held up
                    if lag == 0:
                        emit_attnv(jc, pTbs.pop(jc))
                for jc in range(TC - lag, TC):
                    emit_attnv(jc, pTbs.pop(jc))
                if ic == 0:
                    # boundary chain: keeps the PE busy across the i0->i1
                    # group switch while DVE drains the evictions
                    emit_qproj(fc4, 1)
                pend_t.append(emit_division(pso, fc4, ic))
        flush_transposes(pend_t)
        if oproj_ic0:
            emit_oproj(oproj_ic0)
        emit_oproj([(tci, oc) for tci in range(4, 8) for oc in range(IC)], tail=True)

    nc.compile()
    return nc


def _get_compiled():
    global _COMPILED
    if _COMPILED is None:
        _COMPILED = _build()
    return _COMPILED


def _host_morpho(morpho_types):
    """nearest-verb index per (b, i) (-1 if batch has no verb) and col bias."""
    mt = np.asarray(morpho_types)
    pos = np.arange(S)
    dist = np.abs(pos[:, None] - pos[None, :]).astype(np.float32)
    nearest = np.empty((B, S), np.float32)
    for b in range(B):
        is_verb = mt[b] == 2
        if not is_verb.any():
            nearest[b] = -1.0
            continue
        dm = np.where(is_verb[None, :], dist, BIG)
        nearest[b] = np.argmin(dm, axis=-1).astype(np.float32)
    cb = (
        np.float32(ROOT_BIAS * 0.5) * (mt == 0)
        + np.float32(SUFFIX_BIAS * 0.3) * (mt == 1)
    ).astype(np.float32)
    return nearest, cb


def _fp8_split(a):
    f8 = ml_dtypes.float8_e4m3
    hi = np.ascontiguousarray(a.astype(f8))
    res = np.ascontiguousarray((a - hi.astype(np.float32)).astype(f8))
    return hi, res


def build_in_maps(hidden_states, morpho_types, Wq, bq, Wk, bk, Wv, bv, Wo, bo):
    # weights are pre-scaled by 32 into fp8's normal range (their raw 0.02
    # scale sits in e4m3 subnormals); 1/32 is folded into the q/k eviction
    # scales and, for the v path, into Wo/32 with bv*32 (the softmax
    # denominator is v-scale-invariant)
    hidden_states = np.ascontiguousarray(np.asarray(hidden_states, np.float32))
    bft = ml_dtypes.bfloat16
    Wq = np.asarray(Wq, np.float32) * np.float32(32.0)
    Wk = np.asarray(Wk, np.float32) * np.float32(32.0)
    Wv = np.asarray(Wv, np.float32) * np.float32(32.0)
    Wo = (np.asarray(Wo, np.float32) / np.float32(32.0)).astype(bft)
    bq = np.asarray(bq, np.float32)
    bk = np.asarray(bk, np.float32)
    bv = np.asarray(bv, np.float32) * np.float32(32.0)

    nearest, cb = _host_morpho(morpho_types)

    in_maps = []
    for c in range(8):
        b, g = c // G, c % G
        fs = slice(g * F, (g + 1) * F)
        x8, xr8 = _fp8_split(hidden_states[b].T)
        wq8, wqr8 = _fp8_split(Wq[:, fs])
        wk8, wkr8 = _fp8_split(Wk[:, fs])
        wv8, wvr8 = _fp8_split(Wv[:, fs])
        consts = np.concatenate(
            [
                cb[b].reshape(TC, P).T,
                bq[fs].reshape(FC, P).T,
                bk[fs].reshape(FC, P).T,
            ],
            axis=1,
        ).astype(np.float32)
        in_maps.append({
            "x": x8, "xr": xr8,
            "wq": wq8, "wqr": wqr8,
            "wk": wk8, "wkr": wkr8,
            "wv": wv8, "wvr": wvr8,
            "wo": np.ascontiguousarray(Wo[fs, :]),
            "consts": np.ascontiguousarray(consts),
            "bv": np.ascontiguousarray(bv[fs]),
            "nearf": nearest[b],
        })
    return in_maps


def kernel(hidden_states, morpho_types, Wq, bq, Wk, bk, Wv, bv, Wo, bo):
    bo = np.asarray(bo, np.float32)
    in_maps = build_in_maps(
        hidden_states, morpho_types, Wq, bq, Wk, bk, Wv, bv, Wo, bo
    )
    nc = _get_compiled()
    res = run_bass_kernel_spmd(nc, in_maps, core_ids=list(range(8)))
    out = np.empty((B, S, H), np.float32)
    for b in range(B):
        out[b] = (
            res.results[2 * b]["z"].astype(np.float32)
            + res.results[2 * b + 1]["z"].astype(np.float32)
            + bo
        )
    return out



# revision 46
# speedup vs baseline: 1.1045x; 1.0799x over previous
"""AgglutinativeAttention Trainium2 kernel.

Full inputs in, full output out. Sharding: 8 cores = (batch b in 0..3) x
(head-group g in 0..1). Each core computes, for its batch b and its 8 heads:
  qT/kT = (x @ W{q,k}[:, gF:(g+1)F])^T   [512 feat, 1024 tok]
  v     =  x @ Wv[:, gF:(g+1)F]          [1024 tok, 512 feat] (+ones col/head)
  per head: sT = scores transposed [j, i] via fp8 DoubleRow matmuls (dithered
  dual quantization, see the qT/kT comment), morpho verb bias via an ebT
  elementwise factor + per-partition activation bias (col bias),
  pT = exp(scale*sT + cb) * ebT, oT = v_aug^T @ pT with a ones row giving the
  softmax denominator, divide, then partial z = o @ Wo[gF:(g+1)F, :].
  Host sums the two per-batch partials + bo.

x and the q/k/v weights travel as fp8 hi+residual pairs (split-fp8
DoubleRow projections at 0.75x the bf16 PE cost), wo and z as bf16. DMA
order is arranged so the first v-proj matmul is gated on a few hundred KB,
not 7MB; warm-up matmuls burn the PE p-state ramp during the initial DMA
wait.
"""

import numpy as np
import ml_dtypes
from contextlib import ExitStack

import concourse.bass as bass
import concourse.mybir as mybir
import concourse.tile as tile
from concourse import bacc
from concourse.bass_utils import run_bass_kernel_spmd

B, S, H = 4, 1024, 1024
NH, HD = 16, 64
G = 2                 # head groups (tensor-parallel factor per batch)
F = H // G            # 512 features per core
HPC = NH // G         # 8 heads per core
SCALE = 1.0 / np.sqrt(HD)
VERB_BIAS, ROOT_BIAS, SUFFIX_BIAS = 2.0, 1.5, 1.2
BIG = np.float32(1e9)

f32 = mybir.dt.float32
f32r = mybir.dt.float32r
bf16 = mybir.dt.bfloat16
i32 = mybir.dt.int32

P = 128
KC = H // P           # 8 contraction chunks for projections
TC = S // P           # 8 token chunks of 128
IC = S // 512         # 2 chunks of 512 (matmul free dim)
FC = F // P           # 4 feature chunks per core

_COMPILED = None


def _build():
    nc = bacc.Bacc("TRN2", target_bir_lowering=False, debug=False, num_devices=8)

    fp8d = mybir.dt.float8e4
    x_d = nc.dram_tensor("x", [H, S], fp8d, kind="ExternalInput").ap()
    xr_d = nc.dram_tensor("xr", [H, S], fp8d, kind="ExternalInput").ap()
    wq_d = nc.dram_tensor("wq", [H, F], fp8d, kind="ExternalInput").ap()
    wqr_d = nc.dram_tensor("wqr", [H, F], fp8d, kind="ExternalInput").ap()
    wk_d = nc.dram_tensor("wk", [H, F], fp8d, kind="ExternalInput").ap()
    wkr_d = nc.dram_tensor("wkr", [H, F], fp8d, kind="ExternalInput").ap()
    wv_d = nc.dram_tensor("wv", [H, F], fp8d, kind="ExternalInput").ap()
    wvr_d = nc.dram_tensor("wvr", [H, F], fp8d, kind="ExternalInput").ap()
    wo_d = nc.dram_tensor("wo", [F, H], bf16, kind="ExternalInput").ap()
    # cb [P, TC] | bq [P, FC] | bk [P, FC] packed host-side into one small
    # transfer: three separate HWDGE generations would serialize ~1.9us ahead
    # of the big weight transfers
    consts_d = nc.dram_tensor("consts", [P, TC + 2 * FC], f32, kind="ExternalInput").ap()
    bv_d = nc.dram_tensor("bv", [F], f32, kind="ExternalInput").ap()
    nearf_d = nc.dram_tensor("nearf", [S], f32, kind="ExternalInput").ap()
    z_d = nc.dram_tensor("z", [S, H], bf16, kind="ExternalOutput").ap()

    with tile.TileContext(nc) as tc, ExitStack() as ctx:
        const = ctx.enter_context(tc.tile_pool(name="const", bufs=1))
        big = ctx.enter_context(tc.tile_pool(name="big", bufs=1))
        ppool = ctx.enter_context(tc.tile_pool(name="ppool", bufs=6))
        rlpool = ctx.enter_context(tc.tile_pool(name="rlpool", bufs=2))
        odpool = ctx.enter_context(tc.tile_pool(name="odpool", bufs=2))
        zpool = ctx.enter_context(tc.tile_pool(name="zpool", bufs=6))
        ps_q = ctx.enter_context(tc.tile_pool(name="ps_q", bufs=2, space="PSUM"))
        ps_s = ctx.enter_context(tc.tile_pool(name="ps_s", bufs=2, space="PSUM"))
        ps_o = ctx.enter_context(tc.tile_pool(name="ps_o", bufs=1, space="PSUM"))

        # ---- small constants via SWDGE (gpsimd) — keeps HWDGE free for the
        # wv/x stream that gates the first matmul. Only the two needed early
        # (bv for the first v eviction, nearf for the ohst chain) go first;
        # the rest are emitted after the ohst loop so their transfers don't
        # sit ahead of the wv/x stream on the DMA engines ----
        near_row = const.tile([1, S], f32, tag="near_row")
        bv_row = const.tile([1, F], f32, tag="bv_row")
        # near first: its broadcast gates the whole ebT chain on DVE, while
        # bv is only needed by the first v eviction ~12us in
        nc.gpsimd.dma_start(near_row[:], nearf_d[None, :])
        nc.gpsimd.dma_start(bv_row[:], bv_d[None, :])
        consts_sb = const.tile([P, TC + 2 * FC], f32, tag="consts_sb")
        cb_sb = consts_sb[:, 0:TC]
        bq_sb = consts_sb[:, TC : TC + FC]
        bk_sb = consts_sb[:, TC + FC : TC + 2 * FC]

        # dithered bias copies for the ACT-side slot-1 evictions:
        # ACT computes Copy(ps*(1-h) + bq*(1-h)) == (ps + bq)*(1-h)
        bq_a = const.tile([P, FC], f32, tag="bq_a")
        bk_a = const.tile([P, FC], f32, tag="bk_a")
        bq_b = const.tile([P, FC], f32, tag="bq_b")
        bk_b = const.tile([P, FC], f32, tag="bk_b")

        iota_i = const.tile([P, KC], i32, tag="iota_i")
        nc.gpsimd.iota(iota_i[:], pattern=[[P, KC]], base=0, channel_multiplier=1)
        iota_f = const.tile([P, KC], f32, tag="iota_f")
        nc.gpsimd.tensor_copy(iota_f[:], iota_i[:])

        near_bc = const.tile([P, S], f32, tag="near_bc")
        nc.gpsimd.partition_broadcast(near_bc[:], near_row[:])
        bv_bc = const.tile([P, F], f32, tag="bv_bc")
        nc.gpsimd.partition_broadcast(bv_bc[:], bv_row[:])

        # q/k live as fp8e4 PAIRS: slot 0 holds fp8(v*(1+2^-4)), slot 1 holds
        # fp8(v*(1-2^-4)) — a half-quantization-cell dither. The DoubleRow
        # score matmul (0.5 cycles/row, half the bf16 PE cost) sums both
        # slots, averaging two anti-phased quantization errors (~2x less fp8
        # noise); the exact deterministic scale (a^2+b^2) plus 1/sqrt(d)
        # folds into the exp's scale operand. q itself is NOT pre-scaled so
        # fp8 sees healthy magnitudes.
        fp8 = mybir.dt.float8e4
        DIT = 1.0 / 16.0
        DSC = (1.0 + DIT) ** 2 + (1.0 - DIT) ** 2
        qT = big.tile([P, FC, 2, S], fp8, tag="qT")
        kT = big.tile([P, FC, 2, S], fp8, tag="kT")
        v_sb = big.tile([P, TC, HPC, 65], bf16, tag="v_sb")
        ones64 = const.tile([P, TC * HPC], f32, tag="ones64")
        nc.vector.memset(ones64[:], 1.0)
        nc.vector.tensor_copy(
            v_sb[:, :, :, 64:65],
            ones64.rearrange("p (a b one) -> p a b one", a=TC, b=HPC, one=1),
        )

        # warm-up matmuls on resident constants: the tensor engine p-state
        # ramps only under continuous execution (LOW->MID->full over ~3us),
        # so burn the ramp on throwaway [128x64 @ 128x32] products while the
        # first wv/x DMAs are still in flight — real matmuls then start at
        # full clock
        wps = ps_q.tile([P, 512], f32, tag="ps_proj", name="warmup")
        for w in range(28):
            nc.tensor.matmul(
                wps[0:64, 0:32],
                ones64[:, 0:64],
                ones64[:, 0:32],
                start=True, stop=True,
            )

        # small consts first on HWDGE (one tiny transfer, needed by the
        # earliest DVE work): cb for the first exp, bq/bk for the evictions
        nc.sync.dma_start(consts_sb[:], consts_d)
        # dithered bias copies on DVE (idle until the first eviction) so they
        # never queue behind anything slow
        nc.vector.tensor_scalar(
            bq_a[:], bq_sb[:], 1.0 + DIT, None, mybir.AluOpType.mult
        )
        nc.vector.tensor_scalar(
            bk_a[:], bk_sb[:], 1.0 + DIT, None, mybir.AluOpType.mult
        )
        nc.vector.tensor_scalar(
            bq_b[:], bq_sb[:], 1.0 - DIT, None, mybir.AluOpType.mult
        )
        nc.vector.tensor_scalar(
            bk_b[:], bk_sb[:], 1.0 - DIT, None, mybir.AluOpType.mult
        )

        # identity for the PE o-transposes at group ends
        ident = const.tile([P, P], bf16, tag="ident")
        iota_r = const.tile([P, P], i32, tag="iota_r")
        nc.gpsimd.iota(iota_r[:], pattern=[[1, P]], base=0, channel_multiplier=0)
        iota_rf = const.tile([P, P], f32, tag="iota_rf")
        nc.gpsimd.tensor_copy(iota_rf[:], iota_r[:])
        iota_c = const.tile([P, 1], i32, tag="iota_c")
        nc.gpsimd.iota(iota_c[:], pattern=[[1, 1]], base=0, channel_multiplier=1)
        iota_cf = const.tile([P, 1], f32, tag="iota_cf")
        nc.gpsimd.tensor_copy(iota_cf[:], iota_c[:])
        nc.gpsimd.tensor_scalar(
            ident[:], iota_rf[:], iota_cf[:], None, mybir.AluOpType.is_equal
        )

        # verb factor (transposed): ebT[p, jc, i] = exp(2 * (jc*128+p == nearest[i]))
        # both passes on DVE, compact and early (DVE is otherwise idle before
        # the first eviction): is_equal -> {0,1} bf16, then a 4x tensor_scalar
        # affine (e^2-1)*x + 1 -> {1, e^2}. Putting pass 1 on Pool spreads
        # the chain over 17us of Pool pacing and starves the eviction work.
        EM1 = float(np.exp(2.0) - 1.0)
        ebT = big.tile([P, TC, S], bf16, tag="ebT")
        ohstage = ctx.enter_context(tc.tile_pool(name="ohstage", bufs=2))
        for jc in range(TC):
            ohst = ohstage.tile([P, S], bf16, tag="ohst")
            nc.vector.tensor_scalar(
                ohst[:], near_bc[:], iota_f[:, jc : jc + 1], None,
                mybir.AluOpType.is_equal,
            )
            nc.vector.tensor_scalar(
                ebT[:, jc, :], ohst[:], EM1, 1.0,
                mybir.AluOpType.mult, mybir.AluOpType.add,
            )

        projpool = ctx.enter_context(tc.tile_pool(name="projpool", bufs=1))
        wq_sb = projpool.tile([P, KC, F], fp8, tag="wq_sb")
        wqr_sb = projpool.tile([P, KC, F], fp8, tag="wqr_sb")
        wk_sb = projpool.tile([P, KC, F], fp8, tag="wk_sb")
        wkr_sb = projpool.tile([P, KC, F], fp8, tag="wkr_sb")
        xTh = []
        xTr = []
        for i in range(IC):
            xthalf = projpool.tile([P, KC, 512], fp8, tag=f"xT{i}", name=f"xT{i}")
            xTh.append(xthalf)
            xrhalf = projpool.tile([P, KC, 512], fp8, tag=f"xR{i}", name=f"xR{i}")
            xTr.append(xrhalf)

        attn2 = ctx.enter_context(tc.tile_pool(name="attn2", bufs=1))
        oT = attn2.tile([P, FC, S], bf16, tag="oT")
        wo_sb = attn2.tile([P, FC, H], bf16, tag="wo_sb")

        # ---- x arrives host-pre-transposed as fp8 hi+residual; projections
        # run as split-fp8 DoubleRow kc-pair chains: x8*w8 + xr*w8 + x8*wr
        # (the dropped xr*wr term is ~quantization-squared). Weights are
        # host-pre-scaled by 32 into fp8's normal range; 1/32 folds into the
        # q/k eviction scales and (for v) into host-side Wo/32 + bv*32 ----
        wv_sb = projpool.tile([P, KC, F], fp8, tag="wv_sb")
        wvr_sb = projpool.tile([P, KC, F], fp8, tag="wvr_sb")

        def emit_vproj(tci_range):
            for tci in tci_range:
                ps = ps_q.tile([P, 512], f32, tag="ps_proj")
                half = tci // 4
                tsl = slice((tci % 4) * P, (tci % 4 + 1) * P)
                k = 0
                # term-major: the wv-only terms run before wvr arrives
                for xs, ws in (
                    (xTh[half], wv_sb),
                    (xTr[half], wv_sb),
                    (xTh[half], wvr_sb),
                ):
                    for kc2 in range(0, KC, 2):
                        nc.tensor.matmul(
                            ps[:],
                            xs[:, kc2 : kc2 + 2, tsl],
                            ws[:, kc2 : kc2 + 2, :],
                            start=(k == 0), stop=(k == 11),
                            perf_mode=mybir.MatmulPerfMode.DoubleRow,
                        )
                        k += 1
                nc.vector.tensor_tensor(
                    v_sb[:, tci, :, 0:64],
                    ps.rearrange("p (h d) -> p h d", d=64),
                    bv_bc.rearrange("p (h d) -> p h d", d=64),
                    mybir.AluOpType.add,
                )

        # DMA chain ordered for earliest attention-stream start: q weights +
        # x half0 (first q/k chains ~7us in), v weights (the v projection
        # fills the kproj DMA wait), k weights, x half1, wo last
        nc.sync.dma_start(wq_sb[:], wq_d.rearrange("(kc p) f -> p kc f", p=P))
        nc.sync.dma_start(wqr_sb[:], wqr_d.rearrange("(kc p) f -> p kc f", p=P))
        nc.sync.dma_start(
            xTh[0][:], x_d[:, 0:512].rearrange("(kc p) s -> p kc s", p=P)
        )
        nc.sync.dma_start(
            xTr[0][:], xr_d[:, 0:512].rearrange("(kc p) s -> p kc s", p=P)
        )
        # v weights BEFORE the k weights: the whole v projection then fills
        # the PE while kproj's inputs are still in flight, so once the
        # scores/exp stream starts nothing DMA-gated sits in front of it
        nc.sync.dma_start(wv_sb[:], wv_d.rearrange("(kc p) f -> p kc f", p=P))
        nc.sync.dma_start(wvr_sb[:], wvr_d.rearrange("(kc p) f -> p kc f", p=P))
        nc.sync.dma_start(wk_sb[:], wk_d.rearrange("(kc p) f -> p kc f", p=P))
        nc.sync.dma_start(wkr_sb[:], wkr_d.rearrange("(kc p) f -> p kc f", p=P))
        nc.sync.dma_start(
            xTh[1][:], x_d[:, 512:1024].rearrange("(kc p) s -> p kc s", p=P)
        )
        nc.sync.dma_start(
            xTr[1][:], xr_d[:, 512:1024].rearrange("(kc p) s -> p kc s", p=P)
        )
        nc.sync.dma_start(wo_sb[:], wo_d.rearrange("(fc p) o -> p fc o", p=P))
        # preload the exp table while the DMA stream runs so the first real
        # activation doesn't eat the 1.3us LoadActFuncSet
        exp_warm = const.tile([1, 1], f32, tag="exp_warm")
        nc.scalar.activation(
            exp_warm[:], ones64[0:1, 0:1], mybir.ActivationFunctionType.Exp
        )
        # p-state keepers: junk DoubleRow matmuls gated on each weight tile's
        # DMA keep the tensor engine's clock ramping through the load phase
        for src in (wq_sb, wqr_sb):
            for _ in range(6):
                wps2 = ps_q.tile([P, 512], f32, tag="ps_proj", name="warm2")
                nc.tensor.matmul(
                    wps2[:],
                    src[:, 0:2, 0:128],
                    src[:, 0:2, 0:512],
                    start=True, stop=True,
                    perf_mode=mybir.MatmulPerfMode.DoubleRow,
                )

        # ---- attention interleaved with q projection, per head pair.
        # Both oc halves of a tci land in one zt tile -> one DMA per z row
        # block (halves the HWDGE/sem slots; the tail is store-latency bound)
        zts = {}

        def emit_oproj(tiles, tail=False):
            for n, (tci, oc) in enumerate(tiles):
                if tail and n % 2 == 1:
                    pszw = ps_s.tile([P, 1024], f32, tag="pssb", name="pszw")
                    psz = pszw[:, 0:512]
                else:
                    psz = ps_q.tile([P, 512], f32, tag="ps_proj")
                for fc in range(FC):
                    nc.tensor.matmul(
                        psz[:],
                        oT[:, fc, tci * P : (tci + 1) * P],
                        wo_sb[:, fc, oc * 512 : (oc + 1) * 512],
                        start=(fc == 0), stop=(fc == FC - 1),
                    )
                if tci not in zts:
                    zts[tci] = zpool.tile([P, H], bf16, tag="zt", name="zt")
                zt = zts[tci]
                # gpsimd cannot read PSUM on real HW — evictions go DVE/ACT
                if tail:
                    nc.scalar.copy(zt[:, oc * 512 : (oc + 1) * 512], psz[:])
                else:
                    nc.vector.tensor_copy(zt[:, oc * 512 : (oc + 1) * 512], psz[:])
                # last two row blocks ship per-oc half-stores so the final
                # DMA after the last eviction is half-sized
                split_store = tail and tci >= 6
                if split_store:
                    nc.sync.dma_start(
                        z_d[tci * P : (tci + 1) * P, oc * 512 : (oc + 1) * 512],
                        zt[:, oc * 512 : (oc + 1) * 512],
                    )
                    if oc == IC - 1:
                        del zts[tci]
                elif oc == IC - 1:
                    nc.sync.dma_start(z_d[tci * P : (tci + 1) * P, :], zt[:])
                    del zts[tci]

        # group-end epilogue, stage 1 (DVE): reciprocal of the denominator
        # column + one broadcast multiply -> o_div [tok, (isub, side, feat)]
        def emit_division(pso, fc4_, ic_):
            rl = rlpool.tile([P, 4, 2, 1], f32, tag="rl")
            nc.vector.reciprocal(rl[:], pso[:, :, :, 64:65])
            o_div = odpool.tile([P, 4, 2, 64], bf16, tag="o_div")
            nc.vector.tensor_tensor(
                o_div[:], pso[:, :, :, 0:64],
                rl.to_broadcast((P, 4, 2, 64)),
                mybir.AluOpType.mult,
            )
            return (o_div, fc4_, ic_)

        # stage 2 (PE + ACT): 4 transposes [tok, (side,feat)] -> [(side,feat),
        # tok] through a ps_proj psum slot, then one eviction into oT
        def flush_transposes(pend_t):
            for o_div, fc4_, ic_ in pend_t:
                tp = ps_q.tile([P, 512], bf16, tag="ps_proj", name="tp")
                for isub in range(4):
                    nc.tensor.transpose(
                        tp[:, isub * P : (isub + 1) * P],
                        o_div[:, isub, :, :],
                        ident[:],
                    )
                nc.scalar.copy(oT[:, fc4_, ic_ * 512 : (ic_ + 1) * 512], tp[:])
            pend_t.clear()

        def emit_qproj(fc, icq):
            ps = ps_q.tile([P, 512], f32, tag="ps_proj")
            fsl = slice(fc * P, (fc + 1) * P)
            k = 0
            for ws, xs in (
                (wq_sb, xTh[icq]),
                (wq_sb, xTr[icq]),
                (wqr_sb, xTh[icq]),
            ):
                for kc2 in range(0, KC, 2):
                    nc.tensor.matmul(
                        ps[:],
                        ws[:, kc2 : kc2 + 2, fsl],
                        xs[:, kc2 : kc2 + 2, :],
                        start=(k == 0), stop=(k == 11),
                        perf_mode=mybir.MatmulPerfMode.DoubleRow,
                    )
                    k += 1
            # psum holds 32*q (host-scaled weights); (1 +- DIT)/32 restores
            # scale and applies the dither in one tensor_scalar each
            nc.vector.tensor_scalar(
                qT[:, fc, 0, icq * 512 : (icq + 1) * 512], ps[:],
                (1.0 + DIT) / 32.0, bq_a[:, fc : fc + 1],
                mybir.AluOpType.mult, mybir.AluOpType.add,
            )
            nc.vector.tensor_scalar(
                qT[:, fc, 1, icq * 512 : (icq + 1) * 512], ps[:],
                (1.0 - DIT) / 32.0, bq_b[:, fc : fc + 1],
                mybir.AluOpType.mult, mybir.AluOpType.add,
            )

        def emit_kproj(fc, ick):
            ps = ps_q.tile([P, 512], f32, tag="ps_proj")
            fsl = slice(fc * P, (fc + 1) * P)
            k = 0
            for ws, xs in (
                (wk_sb, xTh[ick]),
                (wk_sb, xTr[ick]),
                (wkr_sb, xTh[ick]),
            ):
                for kc2 in range(0, KC, 2):
                    nc.tensor.matmul(
                        ps[:],
                        ws[:, kc2 : kc2 + 2, fsl],
                        xs[:, kc2 : kc2 + 2, :],
                        start=(k == 0), stop=(k == 11),
                        perf_mode=mybir.MatmulPerfMode.DoubleRow,
                    )
                    k += 1
            nc.vector.tensor_scalar(
                kT[:, fc, 0, ick * 512 : (ick + 1) * 512], ps[:],
                (1.0 + DIT) / 32.0, bk_a[:, fc : fc + 1],
                mybir.AluOpType.mult, mybir.AluOpType.add,
            )
            nc.scalar.activation(
                kT[:, fc, 1, ick * 512 : (ick + 1) * 512], ps[:],
                mybir.ActivationFunctionType.Identity,
                bias=bk_b[:, fc : fc + 1], scale=(1.0 - DIT) / 32.0,
            )

        pend_t = []
        oproj_ic0 = [(tci, oc) for tci in range(4) for oc in range(IC)]
        for fc4 in range(FC):
            if fc4 == 0:
                emit_qproj(0, 0)
                # v(0,1) between qproj and kproj (their wv inputs land before
                # the k weights); v(2..7) drip through the first group's
                # slots, by which time all v DMA is resident
                emit_vproj([0, 1])
                emit_kproj(0, 0)
            for ic in range(IC):
                # heads of the pair interleaved: PE alternates A/B matmuls
                # while ACT/DVE process the other head's exp / verb multiply
                last_group = (fc4 == FC - 1 and ic == IC - 1)
                # flipped attn@v: out [tok(i), 65] with p as the stationary
                # operand puts all 128 PE output partitions to work (the old
                # [65, tok] orientation used 65 of 128) and lands the softmax
                # denominator on the partition axis where the division is one
                # broadcast multiply. [P, 4(isub), 2(side), 128] keeps every
                # accumulation chunk 512B-aligned inside the 2 psum banks.
                pso = ps_o.tile([P, 4, 2, 128], f32, tag="pso", name="pso")
                def emit_attnv(jc, pTb):
                    for isub in range(4):
                        for side in range(2):
                            h = 2 * fc4 + side
                            # psum start=True resets the WHOLE bank: only the
                            # first chain per bank (isub 0/2, side 0) carries
                            # it; the reset zeroes the sibling regions so the
                            # other chains accumulate from there (start=False)
                            nc.tensor.matmul(
                                pso[:, isub, side, 0:65],
                                pTb[:, side * 512 + isub * P : side * 512 + (isub + 1) * P],
                                v_sb[:, jc, h, 0:65],
                                start=(jc == 0 and side == 0 and isub % 2 == 0),
                                stop=(jc == TC - 1),
                            )

                # the first group runs attn@v two slots behind its scores so
                # each attn@v trails its v-projection drip by two slots
                lag = 2 if (fc4 == 0 and ic == 0) else 0
                pTbs = {}

                def emit_slot_top(jc):
                    # this jc's scores + exp first: the exp fires early in the
                    # slot so the ACT stream never waits on the fill work below
                    pssb = ps_s.tile([P, 1024], f32, tag="pssb")
                    for side in range(2):
                        hb = side * 64
                        nc.tensor.matmul(
                            pssb[:, side * 512 : (side + 1) * 512],
                            kT[hb : hb + 64, fc4, 0:2, jc * P : (jc + 1) * P],
                            qT[hb : hb + 64, fc4, 0:2, ic * 512 : (ic + 1) * 512],
                            start=True, stop=True,
                            perf_mode=mybir.MatmulPerfMode.DoubleRow,
                        )
                    pTb = ppool.tile([P, 1024], bf16, tag="pTb")
                    nc.scalar.activation(
                        pTb[:], pssb[:], mybir.ActivationFunctionType.Exp,
                        bias=cb_sb[:, jc : jc + 1], scale=SCALE / DSC,
                    )
                    ebsl = ebT[:, jc, ic * 512 : (ic + 1) * 512]
                    nc.vector.tensor_tensor(
                        pTb.rearrange("p (two n) -> p two n", two=2),
                        pTb.rearrange("p (two n) -> p two n", two=2),
                        ebsl[:, None, :].to_broadcast((P, 2, 512)),
                        mybir.AluOpType.mult,
                    )
                    pTbs[jc] = pTb

                for jc in range(TC):
                    emit_slot_top(jc)
                    # ---- slot fills: projection chains, transposes, o_proj
                    # drips — all behind the slot's scores/exp so the ACT
                    # cadence never blocks on them
                    if fc4 == 0 and ic == 0 and jc < TC - 2:
                        emit_vproj([jc + 2])
                    if ic == 0 and jc == 2:
                        # second-half k projection rides inside the i0 group
                        # (term-major: for fc0 it starts on x half1 arrival,
                        # just ahead of the jc4 scores that need its output)
                        emit_kproj(fc4, 1)
                    if ic == 1 and fc4 < FC - 1:
                        # next head-pair's first-half projections fill the
                        # otherwise-bare i1 groups (needed a full group later)
                        if jc == 2:
                            emit_qproj(fc4 + 1, 0)
                        if jc == 5:
                            emit_kproj(fc4 + 1, 0)
                    if jc == 2 and pend_t:
                        # previous group's o_div is ready by now (its division
                        # ran on DVE at the group boundary) — the transposes
                        # slot into the PE stream without a sem stall
                        flush_transposes(pend_t)
                    if last_group and jc >= 3 and oproj_ic0:
                        # (f3,i0)'s oT lands via the jc2 transpose flush just
                        # above; drip its o_proj tiles through the final group
                        emit_oproj(oproj_ic0[:1])
                        del oproj_ic0[:1]
                    # attn@v at the bottom (lagged in the first group): by now
                    # the DVE verb multiply for its jc is done, so it issues
                    # cleanly and the next slot's scores are never held up
                    if jc >= lag:
                        emit_attnv(jc - lag, pTbs.pop(jc - lag))
                for jc in range(TC - lag, TC):
                    emit_attnv(jc, pTbs.pop(jc))
                if ic == 0:
                    # boundary chain: keeps the PE busy across the i0->i1
                    # group switch while DVE drains the evictions
                    emit_qproj(fc4, 1)
                pend_t.append(emit_division(pso, fc4, ic))
        flush_transposes(pend_t)
        if oproj_ic0:
            emit_oproj(oproj_ic0)
        emit_oproj([(tci, oc) for tci in range(4, 8) for oc in range(IC)], tail=True)

    nc.compile()
    return nc


def _get_compiled():
    global _COMPILED
    if _COMPILED is None:
        _COMPILED = _build()
    return _COMPILED


def _host_morpho(morpho_types):
    """nearest-verb index per (b, i) (-1 if batch has no verb) and col bias."""
    mt = np.asarray(morpho_types)
    pos = np.arange(S)
    dist = np.abs(pos[:, None] - pos[None, :]).astype(np.float32)
    nearest = np.empty((B, S), np.float32)
    for b in range(B):
        is_verb = mt[b] == 2
        if not is_verb.any():
            nearest[b] = -1.0
            continue
        dm = np.where(is_verb[None, :], dist, BIG)
        nearest[b] = np.argmin(dm, axis=-1).astype(np.float32)
    cb = (
        np.float32(ROOT_BIAS * 0.5) * (mt == 0)
        + np.float32(SUFFIX_BIAS * 0.3) * (mt == 1)
    ).astype(np.float32)
    return nearest, cb


def _fp8_split(a):
    f8 = ml_dtypes.float8_e4m3
    hi = np.ascontiguousarray(a.astype(f8))
    res = np.ascontiguousarray((a - hi.astype(np.float32)).astype(f8))
    return hi, res


def build_in_maps(hidden_states, morpho_types, Wq, bq, Wk, bk, Wv, bv, Wo, bo):
    # weights are pre-scaled by 32 into fp8's normal range (their raw 0.02
    # scale sits in e4m3 subnormals); 1/32 is folded into the q/k eviction
    # scales and, for the v path, into Wo/32 with bv*32 (the softmax
    # denominator is v-scale-invariant)
    hidden_states = np.ascontiguousarray(np.asarray(hidden_states, np.float32))
    bft = ml_dtypes.bfloat16
    Wq = np.asarray(Wq, np.float32) * np.float32(32.0)
    Wk = np.asarray(Wk, np.float32) * np.float32(32.0)
    Wv = np.asarray(Wv, np.float32) * np.float32(32.0)
    Wo = (np.asarray(Wo, np.float32) / np.float32(32.0)).astype(bft)
    bq = np.asarray(bq, np.float32)
    bk = np.asarray(bk, np.float32)
    bv = np.asarray(bv, np.float32) * np.float32(32.0)

    nearest, cb = _host_morpho(morpho_types)

    in_maps = []
    for c in range(8):
        b, g = c // G, c % G
        fs = slice(g * F, (g + 1) * F)
        x8, xr8 = _fp8_split(hidden_states[b].T)
        wq8, wqr8 = _fp8_split(Wq[:, fs])
        wk8, wkr8 = _fp8_split(Wk[:, fs])
        wv8, wvr8 = _fp8_split(Wv[:, fs])
        consts = np.concatenate(
            [
                cb[b].reshape(TC, P).T,
                bq[fs].reshape(FC, P).T,
                bk[fs].reshape(FC, P).T,
            ],
            axis=1,
        ).astype(np.float32)
        in_maps.append({
            "x": x8, "xr": xr8,
            "wq": wq8, "wqr": wqr8,
            "wk": wk8, "wkr": wkr8,
            "wv": wv8, "wvr": wvr8,
            "wo": np.ascontiguousarray(Wo[fs, :]),
            "consts": np.ascontiguousarray(consts),
            "bv": np.ascontiguousarray(bv[fs]),
            "nearf": nearest[b],
        })
    return in_maps


def kernel(hidden_states, morpho_types, Wq, bq, Wk, bk, Wv, bv, Wo, bo):
    bo = np.asarray(bo, np.float32)
    in_maps = build_in_maps(
        hidden_states, morpho_types, Wq, bq, Wk, bk, Wv, bv, Wo, bo
    )
    nc = _get_compiled()
    res = run_bass_kernel_spmd(nc, in_maps, core_ids=list(range(8)))
    out = np.empty((B, S, H), np.float32)
    for b in range(B):
        out[b] = (
            res.results[2 * b]["z"].astype(np.float32)
            + res.results[2 * b + 1]["z"].astype(np.float32)
            + bo
        )
    return out

